# revision 6
# baseline (speedup 1.0000x reference)
"""AdaptiveLinearWithChannel: per-channel complex matmul with hypernet rank-2
residual, sharded channel-parallel across 8 TRN2 NeuronCores.

out[c] = x[c] @ (W[model_idx,c] + u_c v_c^T) + bias[model_idx,c] + hyper_shift[c]
  x: (C=32, P=8192, D=128) complex; W_eff: (C, D, D) complex.

Host: tiny hypernet MLPs (1->10->10->{8D,2D}) + rank-2 residual -> W_eff, shift.
Device (per core, 4 channels): for each 128-row chunk of x:
  PE-transpose x_r/x_i chunks (fp32) -> PSUM, cast to bf16 in the PSUM->SBUF
  copy (ACT), then two accumulating bf16 matmuls with moving operands
  [Wr|Wi] and [-Wi|Wr] (N=256) -> psum = [out_r | out_i]; DVE epilogue adds
  (bias + hyper_shift) and interleaves halves into complex64 memory layout.
"""

import sys

sys.path.insert(0, "/opt/trn_rl_repo")

import numpy as np

C, P, D = 32, 8192, 128
N_CORES = 8
CH = C // N_CORES  # channels per core
NSUB = 16          # 128-row chunks per super-chunk (DMA batch)
NJ = P // (128 * NSUB)  # super-chunks per channel

_NC_CACHE = {}


def _build_nc():
    from concourse import bacc, masks, mybir
    from concourse.tile import TileContext

    f32 = mybir.dt.float32
    bf16 = mybir.dt.bfloat16

    nc = bacc.Bacc()
    x_r = nc.declare_dram_parameter("x_r", [CH, P, D], f32, isOutput=False)
    x_i = nc.declare_dram_parameter("x_i", [CH, P, D], f32, isOutput=False)
    wmov = nc.declare_dram_parameter("wmov", [CH, 2, D, 2 * D], f32, isOutput=False)
    shift = nc.declare_dram_parameter("shift", [CH, D, 2 * D], f32, isOutput=False)
    out = nc.declare_dram_parameter("out", [CH, P, 2 * D], f32, isOutput=True)

    RS = 128 * NSUB  # rows per super-chunk

    with TileContext(nc) as tc:
        with (
            tc.tile_pool(name="const", bufs=1) as cpool,
            tc.tile_pool(name="xin", bufs=3) as xpool,
            tc.tile_pool(name="xt", bufs=4) as xtpool,
            tc.tile_pool(name="ptp", bufs=2, space="PSUM") as ptpool,
            tc.tile_pool(name="pop", bufs=4, space="PSUM") as popool,
            tc.tile_pool(name="oout", bufs=3) as opool,
        ):
            ident = cpool.tile([128, 128], f32, tag="ident")
            masks.make_identity(nc, ident[:])

            # weights: DMA f32 staging, cast once to bf16
            w_f32 = cpool.tile([128, CH, 2, 2 * D], f32, tag="wf32")
            nc.sync.dma_start(
                out=w_f32[:], in_=wmov.rearrange("c k d n -> d c k n")
            )
            w_bf = cpool.tile([128, CH, 2, 2 * D], bf16, tag="wbf")
            nc.vector.tensor_copy(w_bf[:], w_f32[:])

            # shift tiles: [p, c, 2D] f32 (host pre-broadcast across partitions)
            shift_sb = cpool.tile([128, CH, 2 * D], f32, tag="shift")
            nc.sync.dma_start(
                out=shift_sb[:], in_=shift.rearrange("c p n -> p c n")
            )

            for c in range(CH):
                w_r_slice = w_bf[:, c, 0, :]
                w_i_slice = w_bf[:, c, 1, :]
                for j in range(NJ):
                    xr_stage = xpool.tile([128, NSUB, D], f32, tag="xr")
                    xi_stage = xpool.tile([128, NSUB, D], f32, tag="xi")
                    rows = x_r[c, j * RS : (j + 1) * RS, :]
                    nc.sync.dma_start(
                        out=xr_stage[:], in_=rows.rearrange("(n p) d -> p n d", p=128)
                    )
                    rows = x_i[c, j * RS : (j + 1) * RS, :]
                    nc.sync.dma_start(
                        out=xi_stage[:], in_=rows.rearrange("(n p) d -> p n d", p=128)
                    )
                    out_sb = opool.tile([128, NSUB, 2 * D], f32, tag="osb")
                    for k in range(NSUB):
                        pt_r = ptpool.tile([128, 128], f32, tag="ptr")
                        nc.tensor.transpose(pt_r[:], xr_stage[:, k, :], ident[:])
                        xt_r = xtpool.tile([128, 128], bf16, tag="xtr")
                        nc.scalar.copy(xt_r[:], pt_r[:])

                        pt_i = ptpool.tile([128, 128], f32, tag="pti")
                        nc.tensor.transpose(pt_i[:], xi_stage[:, k, :], ident[:])
                        xt_i = xtpool.tile([128, 128], bf16, tag="xti")
                        nc.scalar.copy(xt_i[:], pt_i[:])

                        po = popool.tile([128, 2 * D], f32, tag="po")
                        nc.tensor.matmul(
                            po[:], xt_r[:], w_r_slice, start=True, stop=False
                        )
                        nc.tensor.matmul(
                            po[:], xt_i[:], w_i_slice, start=False, stop=True
                        )
                        # epilogue: add shift, interleave halves to complex layout
                        nc.vector.tensor_add(
                            out_sb[:, k, 0 : 2 * D : 2],
                            po[:, 0:D],
                            shift_sb[:, c, 0:D],
                        )
                        nc.vector.tensor_add(
                            out_sb[:, k, 1 : 2 * D : 2],
                            po[:, D : 2 * D],
                            shift_sb[:, c, D : 2 * D],
                        )
                    orows = out[c, j * RS : (j + 1) * RS, :]
                    nc.sync.dma_start(
                        out=orows.rearrange("(n p) m -> p n m", p=128), in_=out_sb[:]
                    )
    nc.compile()
    return nc


def _host_prep(inputs):
    """Hypernet MLPs + rank-2 residual on host (float64), -> per-core arrays."""

    def relu(a):
        return np.maximum(a, 0.0)

    t = np.asarray(inputs["t"], np.float64)  # (1, 1)
    idx = np.asarray(inputs["indices"])

    def hyper(W1, b1, W2, b2, W3, b3):
        W1, b1, W2, b2, W3, b3 = (
            np.asarray(p, np.float64)[idx] for p in (W1, b1, W2, b2, W3, b3)
        )
        h = relu(np.einsum("ti,cio->cto", t, W1) + b1[:, None, :])
        h = relu(np.einsum("cti,cio->cto", h, W2) + b2[:, None, :])
        return np.einsum("cti,cio->cto", h, W3) + b3[:, None, :]

    uv = hyper(*(inputs[k] for k in ("gW1", "gb1", "gW2", "gb2", "gW3", "gb3")))
    uv = uv[:, 0, :]  # (C, 8D)  (nt == 1)
    u = (uv[:, : 2 * D] + 1j * uv[:, 2 * D : 4 * D]).reshape(C, D, 2)
    v = (uv[:, 4 * D : 6 * D] + 1j * uv[:, 6 * D :]).reshape(C, D, 2)
    residual = u @ np.swapaxes(v, -1, -2)  # (C, D, D)

    mi = int(np.asarray(inputs["model_idx"]))
    weight = np.asarray(inputs["weight"], np.float64)
    bias = np.asarray(inputs["bias"], np.float64)
    w = weight[mi, ..., 0] + 1j * weight[mi, ..., 1]  # (C, D, D)
    b = bias[mi, ..., 0] + 1j * bias[mi, ..., 1]  # (C, 1, D)

    W_eff = w + residual  # (C, D, D)

    hs = hyper(*(inputs[k] for k in ("sW1", "sb1", "sW2", "sb2", "sW3", "sb3")))
    hs = hs[:, 0, :]  # (C, 2D)
    shift = b[:, 0, :] + (hs[:, :D] + 1j * hs[:, D:])  # (C, D)

    Wr = np.ascontiguousarray(W_eff.real, dtype=np.float32)
    Wi = np.ascontiguousarray(W_eff.imag, dtype=np.float32)

    # moving operands: [c, 0] = [Wr | Wi], [c, 1] = [-Wi | Wr]
    wmov = np.empty((C, 2, D, 2 * D), np.float32)
    wmov[:, 0, :, :D] = Wr
    wmov[:, 0, :, D:] = Wi
    wmov[:, 1, :, :D] = -Wi
    wmov[:, 1, :, D:] = Wr

    # shift tile, broadcast across 128 partitions: [c, p, 0:D]=re, [c, p, D:]=im
    shift_t = np.empty((C, D, 2 * D), np.float32)
    shift_t[:, :, :D] = shift.real.astype(np.float32)[:, None, :]
    shift_t[:, :, D:] = shift.imag.astype(np.float32)[:, None, :]

    x_r = np.ascontiguousarray(inputs["x_real"], np.float32)
    x_i = np.ascontiguousarray(inputs["x_imag"], np.float32)

    in_maps = []
    for core in range(N_CORES):
        c0 = core * CH
        in_maps.append(
            {
                "x_r": x_r[c0 : c0 + CH],
                "x_i": x_i[c0 : c0 + CH],
                "wmov": np.ascontiguousarray(wmov[c0 : c0 + CH]),
                "shift": np.ascontiguousarray(shift_t[c0 : c0 + CH]),
            }
        )
    return in_maps


def _get_nc():
    if "nc" not in _NC_CACHE:
        _NC_CACHE["nc"] = _build_nc()
    return _NC_CACHE["nc"]


def kernel(**inputs):
    from concourse.bass_utils import run_bass_kernel_spmd

    nc = _get_nc()
    in_maps = _host_prep(inputs)
    res = run_bass_kernel_spmd(nc, in_maps, core_ids=list(range(N_CORES)))
    outs = [res.results[i]["out"] for i in range(N_CORES)]
    full = np.concatenate(outs, axis=0)  # (C, P, 2D) f32 interleaved
    cplx = np.ascontiguousarray(full).view(np.complex64)  # (C, P, D)
    return cplx[None]  # (1, C, P, D) complex64


# revision 7
# speedup vs baseline: 1.8556x; 1.8556x over previous
"""AdaptiveLinearWithChannel: per-channel complex matmul with hypernet rank-2
residual, sharded channel-parallel across 8 TRN2 NeuronCores.

out[c] = x[c] @ (W[model_idx,c] + u_c v_c^T) + bias[model_idx,c] + hyper_shift[c]
  x: (C=32, P=8192, D=128) complex; W_eff: (C, D, D) complex.

Host: tiny hypernet MLPs (1->10->10->{8D,2D}) + rank-2 residual -> W_eff and
combined shift (float64), then cast x to bf16 and pre-transpose to (C, D, P)
so the device needs no on-chip transposes and half the DMA bytes. The matmul
consumes bf16 either way, so the input cast adds no error vs casting on-chip.

Device (per core, 4 channels): for each 128-row chunk of x, two accumulating
bf16 matmuls with stationary xT chunks and moving operands [Wr|Wi] and
[-Wi|Wr] (N=256) -> psum = [out_r | out_i]; DVE epilogue adds
(bias + hyper_shift) and interleaves halves into complex element order,
writing bf16. Host widens bf16 -> complex64.
"""

import sys

sys.path.insert(0, "/opt/trn_rl_repo")

import numpy as np

C, P, D = 32, 8192, 128
N_CORES = 8
CH = C // N_CORES  # channels per core
PSUB = 2048        # p-columns per DMA slab
NCHUNK = PSUB // 128
NJ = P // PSUB     # slabs per channel

_NC_CACHE = {}


def _build_nc():
    from concourse import bacc, mybir
    from concourse.tile import TileContext

    f32 = mybir.dt.float32
    bf16 = mybir.dt.bfloat16

    nc = bacc.Bacc()
    xt_r = nc.declare_dram_parameter("xt_r", [CH, D, P], bf16, isOutput=False)
    xt_i = nc.declare_dram_parameter("xt_i", [CH, D, P], bf16, isOutput=False)
    wmov = nc.declare_dram_parameter("wmov", [CH, 2, D, 2 * D], bf16, isOutput=False)
    shift = nc.declare_dram_parameter("shift", [CH, D, 2 * D], f32, isOutput=False)
    out = nc.declare_dram_parameter("out", [CH, P, 2 * D], bf16, isOutput=True)

    with TileContext(nc) as tc:
        with (
            tc.tile_pool(name="const", bufs=1) as cpool,
            tc.tile_pool(name="xin", bufs=3) as xpool,
            tc.tile_pool(name="pop", bufs=8, space="PSUM") as popool,
            tc.tile_pool(name="oout", bufs=3) as opool,
        ):
            w_bf = cpool.tile([128, CH, 2, 2 * D], bf16, tag="wbf")
            nc.sync.dma_start(out=w_bf[:], in_=wmov.rearrange("c k d n -> d c k n"))

            # shift tiles: [p, c, 2D] f32 (host pre-broadcast across partitions)
            shift_sb = cpool.tile([128, CH, 2 * D], f32, tag="shift")
            nc.sync.dma_start(out=shift_sb[:], in_=shift.rearrange("c p n -> p c n"))

            for c in range(CH):
                w_r_slice = w_bf[:, c, 0, :]
                w_i_slice = w_bf[:, c, 1, :]
                for j in range(NJ):
                    xr_slab = xpool.tile([128, PSUB], bf16, tag="xr")
                    xi_slab = xpool.tile([128, PSUB], bf16, tag="xi")
                    nc.sync.dma_start(
                        out=xr_slab[:], in_=xt_r[c, :, j * PSUB : (j + 1) * PSUB]
                    )
                    nc.sync.dma_start(
                        out=xi_slab[:], in_=xt_i[c, :, j * PSUB : (j + 1) * PSUB]
                    )
                    out_sb = opool.tile([128, NCHUNK, 2 * D], bf16, tag="osb")
                    for k in range(NCHUNK):
                        po = popool.tile([128, 2 * D], f32, tag="po")
                        nc.tensor.matmul(
                            po[:],
                            xr_slab[:, k * 128 : (k + 1) * 128],
                            w_r_slice,
                            start=True,
                            stop=False,
                        )
                        nc.tensor.matmul(
                            po[:],
                            xi_slab[:, k * 128 : (k + 1) * 128],
                            w_i_slice,
                            start=False,
                            stop=True,
                        )
                        # epilogue: add shift, interleave halves to complex order
                        nc.vector.tensor_add(
                            out_sb[:, k, 0 : 2 * D : 2],
                            po[:, 0:D],
                            shift_sb[:, c, 0:D],
                        )
                        nc.vector.tensor_add(
                            out_sb[:, k, 1 : 2 * D : 2],
                            po[:, D : 2 * D],
                            shift_sb[:, c, D : 2 * D],
                        )
                    orows = out[c, j * PSUB : (j + 1) * PSUB, :]
                    nc.sync.dma_start(
                        out=orows.rearrange("(n p) m -> p n m", p=128), in_=out_sb[:]
                    )
    nc.compile()
    return nc


def _host_prep(inputs):
    """Hypernet MLPs + rank-2 residual on host (float64), -> per-core arrays."""
    import ml_dtypes

    bf16 = ml_dtypes.bfloat16

    def relu(a):
        return np.maximum(a, 0.0)

    t = np.asarray(inputs["t"], np.float64)  # (1, 1)
    idx = np.asarray(inputs["indices"])

    def hyper(W1, b1, W2, b2, W3, b3):
        W1, b1, W2, b2, W3, b3 = (
            np.asarray(p, np.float64)[idx] for p in (W1, b1, W2, b2, W3, b3)
        )
        h = relu(np.einsum("ti,cio->cto", t, W1) + b1[:, None, :])
        h = relu(np.einsum("cti,cio->cto", h, W2) + b2[:, None, :])
        return np.einsum("cti,cio->cto", h, W3) + b3[:, None, :]

    uv = hyper(*(inputs[k] for k in ("gW1", "gb1", "gW2", "gb2", "gW3", "gb3")))
    uv = uv[:, 0, :]  # (C, 8D)  (nt == 1)
    u = (uv[:, : 2 * D] + 1j * uv[:, 2 * D : 4 * D]).reshape(C, D, 2)
    v = (uv[:, 4 * D : 6 * D] + 1j * uv[:, 6 * D :]).reshape(C, D, 2)
    residual = u @ np.swapaxes(v, -1, -2)  # (C, D, D)

    mi = int(np.asarray(inputs["model_idx"]))
    weight = np.asarray(inputs["weight"], np.float64)
    bias = np.asarray(inputs["bias"], np.float64)
    w = weight[mi, ..., 0] + 1j * weight[mi, ..., 1]  # (C, D, D)
    b = bias[mi, ..., 0] + 1j * bias[mi, ..., 1]  # (C, 1, D)

    W_eff = w + residual  # (C, D, D)

    hs = hyper(*(inputs[k] for k in ("sW1", "sb1", "sW2", "sb2", "sW3", "sb3")))
    hs = hs[:, 0, :]  # (C, 2D)
    shift = b[:, 0, :] + (hs[:, :D] + 1j * hs[:, D:])  # (C, D)

    Wr = W_eff.real.astype(np.float32)
    Wi = W_eff.imag.astype(np.float32)

    # moving operands: [c, 0] = [Wr | Wi], [c, 1] = [-Wi | Wr]
    wmov = np.empty((C, 2, D, 2 * D), np.float32)
    wmov[:, 0, :, :D] = Wr
    wmov[:, 0, :, D:] = Wi
    wmov[:, 1, :, :D] = -Wi
    wmov[:, 1, :, D:] = Wr
    wmov = wmov.astype(bf16)

    # shift tile, broadcast across 128 partitions: [c, p, 0:D]=re, [c, p, D:]=im
    shift_t = np.empty((C, D, 2 * D), np.float32)
    shift_t[:, :, :D] = shift.real.astype(np.float32)[:, None, :]
    shift_t[:, :, D:] = shift.imag.astype(np.float32)[:, None, :]

    # x: cast to bf16 and transpose to (C, D, P) so device needs no transposes
    xt_r = np.ascontiguousarray(
        np.asarray(inputs["x_real"], np.float32).transpose(0, 2, 1)
    ).astype(bf16)
    xt_i = np.ascontiguousarray(
        np.asarray(inputs["x_imag"], np.float32).transpose(0, 2, 1)
    ).astype(bf16)

    in_maps = []
    for core in range(N_CORES):
        c0 = core * CH
        in_maps.append(
            {
                "xt_r": xt_r[c0 : c0 + CH],
                "xt_i": xt_i[c0 : c0 + CH],
                "wmov": np.ascontiguousarray(wmov[c0 : c0 + CH]),
                "shift": np.ascontiguousarray(shift_t[c0 : c0 + CH]),
            }
        )
    return in_maps


def _assemble(outs):
    """bf16 interleaved (CH, P, 2D) per core -> (1, C, P, D) complex64."""
    full = np.concatenate(outs, axis=0)  # (C, P, 2D) bf16
    u32 = full.view(np.uint16).astype(np.uint32) << 16
    f32 = u32.view(np.float32)
    return np.ascontiguousarray(f32).view(np.complex64)[None]


def _get_nc():
    if "nc" not in _NC_CACHE:
        _NC_CACHE["nc"] = _build_nc()
    return _NC_CACHE["nc"]


def kernel(**inputs):
    from concourse.bass_utils import run_bass_kernel_spmd

    nc = _get_nc()
    in_maps = _host_prep(inputs)
    res = run_bass_kernel_spmd(nc, in_maps, core_ids=list(range(N_CORES)))
    return _assemble([res.results[i]["out"] for i in range(N_CORES)])


# revision 12
# speedup vs baseline: 1.9548x; 1.0535x over previous
"""AdaptiveLinearWithChannel: per-channel complex matmul with hypernet rank-2
residual, sharded channel-parallel across 8 TRN2 NeuronCores.

out[c] = x[c] @ (W[model_idx,c] + u_c v_c^T) + bias[model_idx,c] + hyper_shift[c]
  x: (C=32, P=8192, D=128) complex; W_eff: (C, D, D) complex.

Host: tiny hypernet MLPs (1->10->10->{8D,2D}) + rank-2 residual -> W_eff and
combined shift (float64), then cast x to bf16 and pre-transpose to (C, D, P)
so the device needs no on-chip transposes and half the DMA bytes. The matmul
consumes bf16 either way, so the input cast adds no error vs casting on-chip.

Device (per core, 4 channels): for each 128-row chunk of x, two accumulating
bf16 matmuls with stationary xT chunks and moving operands [Wr|Wi] and
[-Wi|Wr] (N=256) -> psum = [out_r | out_i]; DVE epilogue adds
(bias + hyper_shift) and interleaves halves into complex element order,
writing bf16. Host widens bf16 -> complex64.
"""

import sys

sys.path.insert(0, "/opt/trn_rl_repo")

import numpy as np

C, P, D = 32, 8192, 128
N_CORES = 8
CH = C // N_CORES  # channels per core
PSUB = 4096        # p-columns per DMA slab
NCHUNK = PSUB // 128
NJ = P // PSUB     # slabs per channel
NB = 4             # 128-chunks batched per PSUM tile / epilogue op

_NC_CACHE = {}


def _build_nc():
    from concourse import bacc, mybir
    from concourse.tile import TileContext

    f32 = mybir.dt.float32
    bf16 = mybir.dt.bfloat16

    nc = bacc.Bacc()
    xt_r = nc.declare_dram_parameter("xt_r", [CH, D, P], bf16, isOutput=False)
    xt_i = nc.declare_dram_parameter("xt_i", [CH, D, P], bf16, isOutput=False)
    wmov = nc.declare_dram_parameter("wmov", [CH, 2, D, 2 * D], bf16, isOutput=False)
    shift = nc.declare_dram_parameter("shift", [CH, D, 2 * D], f32, isOutput=False)
    out = nc.declare_dram_parameter("out", [CH, P, 2 * D], bf16, isOutput=True)

    with TileContext(nc) as tc:
        with (
            tc.tile_pool(name="const", bufs=1) as cpool,
            tc.tile_pool(name="xin", bufs=3) as xpool,
            tc.tile_pool(name="pop", bufs=4, space="PSUM") as popool,
            tc.tile_pool(name="oout", bufs=3) as opool,
        ):
            w_bf = cpool.tile([128, CH, 2, 2 * D], bf16, tag="wbf")
            nc.sync.dma_start(out=w_bf[:], in_=wmov.rearrange("c k d n -> d c k n"))

            # shift tiles: [p, c, 2D] f32 (host pre-broadcast across partitions)
            shift_sb = cpool.tile([128, CH, 2 * D], f32, tag="shift")
            nc.sync.dma_start(out=shift_sb[:], in_=shift.rearrange("c p n -> p c n"))

            for c in range(CH):
                w_r_slice = w_bf[:, c, 0, :]
                w_i_slice = w_bf[:, c, 1, :]
                for j in range(NJ):
                    xr_slab = xpool.tile([128, PSUB], bf16, tag="xr")
                    xi_slab = xpool.tile([128, PSUB], bf16, tag="xi")
                    nc.sync.dma_start(
                        out=xr_slab[:], in_=xt_r[c, :, j * PSUB : (j + 1) * PSUB]
                    )
                    nc.sync.dma_start(
                        out=xi_slab[:], in_=xt_i[c, :, j * PSUB : (j + 1) * PSUB]
                    )
                    out_sb = opool.tile([128, NCHUNK, 2 * D], bf16, tag="osb")
                    for k0 in range(0, NCHUNK, NB):
                        po = popool.tile([128, NB, 2 * D], f32, tag="po")
                        for b in range(NB):
                            k = k0 + b
                            nc.tensor.matmul(
                                po[:, b, :],
                                xr_slab[:, k * 128 : (k + 1) * 128],
                                w_r_slice,
                                start=True,
                                stop=False,
                            )
                            nc.tensor.matmul(
                                po[:, b, :],
                                xi_slab[:, k * 128 : (k + 1) * 128],
                                w_i_slice,
                                start=False,
                                stop=True,
                            )
                        # epilogue: add shift, interleave halves to complex order
                        nc.vector.tensor_add(
                            out_sb[:, k0 : k0 + NB, 0 : 2 * D : 2],
                            po[:, :, 0:D],
                            shift_sb[:, c : c + 1, 0:D].broadcast_to([128, NB, D]),
                        )
                        nc.vector.tensor_add(
                            out_sb[:, k0 : k0 + NB, 1 : 2 * D : 2],
                            po[:, :, D : 2 * D],
                            shift_sb[:, c : c + 1, D : 2 * D].broadcast_to(
                                [128, NB, D]
                            ),
                        )
                    orows = out[c, j * PSUB : (j + 1) * PSUB, :]
                    nc.scalar.dma_start(
                        out=orows.rearrange("(n p) m -> p n m", p=128), in_=out_sb[:]
                    )
    nc.compile()
    return nc


def _host_prep(inputs):
    """Hypernet MLPs + rank-2 residual on host (float64), -> per-core arrays."""
    import ml_dtypes

    bf16 = ml_dtypes.bfloat16

    def relu(a):
        return np.maximum(a, 0.0)

    t = np.asarray(inputs["t"], np.float64)  # (1, 1)
    idx = np.asarray(inputs["indices"])

    def hyper(W1, b1, W2, b2, W3, b3):
        W1, b1, W2, b2, W3, b3 = (
            np.asarray(p, np.float64)[idx] for p in (W1, b1, W2, b2, W3, b3)
        )
        h = relu(np.einsum("ti,cio->cto", t, W1) + b1[:, None, :])
        h = relu(np.einsum("cti,cio->cto", h, W2) + b2[:, None, :])
        return np.einsum("cti,cio->cto", h, W3) + b3[:, None, :]

    uv = hyper(*(inputs[k] for k in ("gW1", "gb1", "gW2", "gb2", "gW3", "gb3")))
    uv = uv[:, 0, :]  # (C, 8D)  (nt == 1)
    u = (uv[:, : 2 * D] + 1j * uv[:, 2 * D : 4 * D]).reshape(C, D, 2)
    v = (uv[:, 4 * D : 6 * D] + 1j * uv[:, 6 * D :]).reshape(C, D, 2)
    residual = u @ np.swapaxes(v, -1, -2)  # (C, D, D)

    mi = int(np.asarray(inputs["model_idx"]))
    weight = np.asarray(inputs["weight"], np.float64)
    bias = np.asarray(inputs["bias"], np.float64)
    w = weight[mi, ..., 0] + 1j * weight[mi, ..., 1]  # (C, D, D)
    b = bias[mi, ..., 0] + 1j * bias[mi, ..., 1]  # (C, 1, D)

    W_eff = w + residual  # (C, D, D)

    hs = hyper(*(inputs[k] for k in ("sW1", "sb1", "sW2", "sb2", "sW3", "sb3")))
    hs = hs[:, 0, :]  # (C, 2D)
    shift = b[:, 0, :] + (hs[:, :D] + 1j * hs[:, D:])  # (C, D)

    Wr = W_eff.real.astype(np.float32)
    Wi = W_eff.imag.astype(np.float32)

    # moving operands: [c, 0] = [Wr | Wi], [c, 1] = [-Wi | Wr]
    wmov = np.empty((C, 2, D, 2 * D), np.float32)
    wmov[:, 0, :, :D] = Wr
    wmov[:, 0, :, D:] = Wi
    wmov[:, 1, :, :D] = -Wi
    wmov[:, 1, :, D:] = Wr
    wmov = wmov.astype(bf16)

    # shift tile, broadcast across 128 partitions: [c, p, 0:D]=re, [c, p, D:]=im
    shift_t = np.empty((C, D, 2 * D), np.float32)
    shift_t[:, :, :D] = shift.real.astype(np.float32)[:, None, :]
    shift_t[:, :, D:] = shift.imag.astype(np.float32)[:, None, :]

    # x: cast to bf16 and transpose to (C, D, P) so device needs no transposes
    xt_r = np.ascontiguousarray(
        np.asarray(inputs["x_real"], np.float32).transpose(0, 2, 1)
    ).astype(bf16)
    xt_i = np.ascontiguousarray(
        np.asarray(inputs["x_imag"], np.float32).transpose(0, 2, 1)
    ).astype(bf16)

    in_maps = []
    for core in range(N_CORES):
        c0 = core * CH
        in_maps.append(
            {
                "xt_r": xt_r[c0 : c0 + CH],
                "xt_i": xt_i[c0 : c0 + CH],
                "wmov": np.ascontiguousarray(wmov[c0 : c0 + CH]),
                "shift": np.ascontiguousarray(shift_t[c0 : c0 + CH]),
            }
        )
    return in_maps


def _assemble(outs):
    """bf16 interleaved (CH, P, 2D) per core -> (1, C, P, D) complex64."""
    full = np.concatenate(outs, axis=0)  # (C, P, 2D) bf16
    u32 = full.view(np.uint16).astype(np.uint32) << 16
    f32 = u32.view(np.float32)
    return np.ascontiguousarray(f32).view(np.complex64)[None]


def _get_nc():
    if "nc" not in _NC_CACHE:
        _NC_CACHE["nc"] = _build_nc()
    return _NC_CACHE["nc"]


def kernel(**inputs):
    from concourse.bass_utils import run_bass_kernel_spmd

    nc = _get_nc()
    in_maps = _host_prep(inputs)
    res = run_bass_kernel_spmd(nc, in_maps, core_ids=list(range(N_CORES)))
    return _assemble([res.results[i]["out"] for i in range(N_CORES)])


# revision 18
# speedup vs baseline: 2.1962x; 1.1235x over previous
"""AdaptiveLinearWithChannel: per-channel complex matmul with hypernet rank-2
residual, sharded channel-parallel across 8 TRN2 NeuronCores.

out[c] = x[c] @ (W[model_idx,c] + u_c v_c^T) + bias[model_idx,c] + hyper_shift[c]
  x: (C=32, P=8192, D=128) complex; W_eff: (C, D, D) complex.

Host: tiny hypernet MLPs (1->10->10->{8D,2D}) + rank-2 residual -> W_eff and
combined shift (float64), then cast x to bf16 and pre-transpose to (C, D, P)
so the device needs no on-chip transposes and half the DMA bytes. The matmul
consumes bf16 either way, so the input cast adds no error vs casting on-chip.

Device (per core, 4 channels): for each 128-row chunk of x, two accumulating
bf16 matmuls with stationary xT chunks and moving operands [Wr|Wi] and
[-Wi|Wr] (N=256) -> psum = [out_r | out_i]; DVE epilogue adds
(bias + hyper_shift) and interleaves halves into complex element order,
writing bf16. Host widens bf16 -> complex64.
"""

import sys

sys.path.insert(0, "/opt/trn_rl_repo")

import numpy as np

C, P, D = 32, 8192, 128
N_CORES = 8
CH = C // N_CORES  # channels per core
PSUB = 4096        # p-columns per DMA slab
NCHUNK = PSUB // 128
NJ = P // PSUB     # slabs per channel
NB = 4             # 128-chunks batched per PSUM tile / epilogue op

_NC_CACHE = {}


def _build_nc():
    from concourse import bacc, mybir
    from concourse.tile import TileContext

    f32 = mybir.dt.float32
    bf16 = mybir.dt.bfloat16

    nc = bacc.Bacc()
    xt_r = nc.declare_dram_parameter("xt_r", [CH, D, P], bf16, isOutput=False)
    xt_i = nc.declare_dram_parameter("xt_i", [CH, D, P], bf16, isOutput=False)
    wmov = nc.declare_dram_parameter("wmov", [CH, 2, D, 2 * D], bf16, isOutput=False)
    shift = nc.declare_dram_parameter("shift", [CH, D, 2 * D], f32, isOutput=False)
    # partition-major output layout: 16KB contiguous runs per partition on the
    # store DMA; host transposes (j, p128, k) -> rows afterwards.
    out = nc.declare_dram_parameter(
        "out", [CH, NJ, 128, NCHUNK, 2 * D], bf16, isOutput=True
    )

    with TileContext(nc) as tc:
        with (
            tc.tile_pool(name="const", bufs=1) as cpool,
            tc.tile_pool(name="xin", bufs=4) as xpool,
            tc.tile_pool(name="pop", bufs=4, space="PSUM") as popool,
            tc.tile_pool(name="oout", bufs=3) as opool,
        ):
            # prologue params via SWDGE so the HWDGE queues start with x slabs
            w_bf = cpool.tile([128, CH, 2, 2 * D], bf16, tag="wbf")
            nc.gpsimd.dma_start(out=w_bf[:], in_=wmov.rearrange("c k d n -> d c k n"))

            # shift tiles: [p, c, 2D] f32 (host pre-broadcast across partitions)
            shift_sb = cpool.tile([128, CH, 2 * D], f32, tag="shift")
            nc.gpsimd.dma_start(
                out=shift_sb[:], in_=shift.rearrange("c p n -> p c n")
            )

            for c in range(CH):
                w_r_slice = w_bf[:, c, 0, :]
                w_i_slice = w_bf[:, c, 1, :]
                for j in range(NJ):
                    xr_slab = xpool.tile([128, PSUB], bf16, tag="xr")
                    xi_slab = xpool.tile([128, PSUB], bf16, tag="xi")
                    nc.sync.dma_start(
                        out=xr_slab[:], in_=xt_r[c, :, j * PSUB : (j + 1) * PSUB]
                    )
                    nc.sync.dma_start(
                        out=xi_slab[:], in_=xt_i[c, :, j * PSUB : (j + 1) * PSUB]
                    )
                    out_sb = opool.tile([128, NCHUNK, 2 * D], bf16, tag="osb")
                    half = NCHUNK // 2
                    for k0 in range(0, NCHUNK, NB):
                        if k0 == half:
                            # store first half while second half computes
                            nc.scalar.dma_start(
                                out=out[c, j, :, 0:half, :],
                                in_=out_sb[:, 0:half, :],
                            )
                        po = popool.tile([128, NB, 2 * D], f32, tag="po")
                        for b in range(NB):
                            k = k0 + b
                            nc.tensor.matmul(
                                po[:, b, :],
                                xr_slab[:, k * 128 : (k + 1) * 128],
                                w_r_slice,
                                start=True,
                                stop=False,
                            )
                            nc.tensor.matmul(
                                po[:, b, :],
                                xi_slab[:, k * 128 : (k + 1) * 128],
                                w_i_slice,
                                start=False,
                                stop=True,
                            )
                        # epilogue: add shift, interleave halves to complex order
                        nc.vector.tensor_add(
                            out_sb[:, k0 : k0 + NB, 0 : 2 * D : 2],
                            po[:, :, 0:D],
                            shift_sb[:, c : c + 1, 0:D].broadcast_to([128, NB, D]),
                        )
                        nc.vector.tensor_add(
                            out_sb[:, k0 : k0 + NB, 1 : 2 * D : 2],
                            po[:, :, D : 2 * D],
                            shift_sb[:, c : c + 1, D : 2 * D].broadcast_to(
                                [128, NB, D]
                            ),
                        )
                    nc.scalar.dma_start(
                        out=out[c, j, :, half:NCHUNK, :],
                        in_=out_sb[:, half:NCHUNK, :],
                    )
    nc.compile()
    return nc


def _host_prep(inputs):
    """Hypernet MLPs + rank-2 residual on host (float64), -> per-core arrays."""
    import ml_dtypes

    bf16 = ml_dtypes.bfloat16

    def relu(a):
        return np.maximum(a, 0.0)

    t = np.asarray(inputs["t"], np.float64)  # (1, 1)
    idx = np.asarray(inputs["indices"])

    def hyper(W1, b1, W2, b2, W3, b3):
        W1, b1, W2, b2, W3, b3 = (
            np.asarray(p, np.float64)[idx] for p in (W1, b1, W2, b2, W3, b3)
        )
        h = relu(np.einsum("ti,cio->cto", t, W1) + b1[:, None, :])
        h = relu(np.einsum("cti,cio->cto", h, W2) + b2[:, None, :])
        return np.einsum("cti,cio->cto", h, W3) + b3[:, None, :]

    uv = hyper(*(inputs[k] for k in ("gW1", "gb1", "gW2", "gb2", "gW3", "gb3")))
    uv = uv[:, 0, :]  # (C, 8D)  (nt == 1)
    u = (uv[:, : 2 * D] + 1j * uv[:, 2 * D : 4 * D]).reshape(C, D, 2)
    v = (uv[:, 4 * D : 6 * D] + 1j * uv[:, 6 * D :]).reshape(C, D, 2)
    residual = u @ np.swapaxes(v, -1, -2)  # (C, D, D)

    mi = int(np.asarray(inputs["model_idx"]))
    weight = np.asarray(inputs["weight"], np.float64)
    bias = np.asarray(inputs["bias"], np.float64)
    w = weight[mi, ..., 0] + 1j * weight[mi, ..., 1]  # (C, D, D)
    b = bias[mi, ..., 0] + 1j * bias[mi, ..., 1]  # (C, 1, D)

    W_eff = w + residual  # (C, D, D)

    hs = hyper(*(inputs[k] for k in ("sW1", "sb1", "sW2", "sb2", "sW3", "sb3")))
    hs = hs[:, 0, :]  # (C, 2D)
    shift = b[:, 0, :] + (hs[:, :D] + 1j * hs[:, D:])  # (C, D)

    Wr = W_eff.real.astype(np.float32)
    Wi = W_eff.imag.astype(np.float32)

    # moving operands: [c, 0] = [Wr | Wi], [c, 1] = [-Wi | Wr]
    wmov = np.empty((C, 2, D, 2 * D), np.float32)
    wmov[:, 0, :, :D] = Wr
    wmov[:, 0, :, D:] = Wi
    wmov[:, 1, :, :D] = -Wi
    wmov[:, 1, :, D:] = Wr
    wmov = wmov.astype(bf16)

    # shift tile, broadcast across 128 partitions: [c, p, 0:D]=re, [c, p, D:]=im
    shift_t = np.empty((C, D, 2 * D), np.float32)
    shift_t[:, :, :D] = shift.real.astype(np.float32)[:, None, :]
    shift_t[:, :, D:] = shift.imag.astype(np.float32)[:, None, :]

    # x: cast to bf16 and transpose to (C, D, P) so device needs no transposes
    xt_r = np.ascontiguousarray(
        np.asarray(inputs["x_real"], np.float32).transpose(0, 2, 1)
    ).astype(bf16)
    xt_i = np.ascontiguousarray(
        np.asarray(inputs["x_imag"], np.float32).transpose(0, 2, 1)
    ).astype(bf16)

    in_maps = []
    for core in range(N_CORES):
        c0 = core * CH
        in_maps.append(
            {
                "xt_r": xt_r[c0 : c0 + CH],
                "xt_i": xt_i[c0 : c0 + CH],
                "wmov": np.ascontiguousarray(wmov[c0 : c0 + CH]),
                "shift": np.ascontiguousarray(shift_t[c0 : c0 + CH]),
            }
        )
    return in_maps


def _assemble(outs):
    """bf16 (CH, NJ, 128, NCHUNK, 2D) per core -> (1, C, P, D) complex64."""
    full = np.concatenate(outs, axis=0)  # (C, NJ, 128, NCHUNK, 2D) bf16
    full = full.transpose(0, 1, 3, 2, 4).reshape(C, P, 2 * D)
    u32 = full.view(np.uint16).astype(np.uint32) << 16
    f32 = u32.view(np.float32)
    return np.ascontiguousarray(f32).view(np.complex64)[None]


def _get_nc():
    if "nc" not in _NC_CACHE:
        _NC_CACHE["nc"] = _build_nc()
    return _NC_CACHE["nc"]


def kernel(**inputs):
    from concourse.bass_utils import run_bass_kernel_spmd

    nc = _get_nc()
    in_maps = _host_prep(inputs)
    res = run_bass_kernel_spmd(nc, in_maps, core_ids=list(range(N_CORES)))
    return _assemble([res.results[i]["out"] for i in range(N_CORES)])
